# revision 19
# baseline (speedup 1.0000x reference)
"""Trainium2 Bass kernel for AttentionInteraction (cross-attention between
ads/cat node blocks of B=512 graphs, data-parallel over 8 NeuronCores).

Self-contained: hardcodes shapes B=512, NA=16, NC=256, D=256, 8 cores.
kernel(**inputs) takes the FULL unsharded inputs and returns
(new_h_ads, new_h_cat) like the reference.
"""

import math
import ml_dtypes
import numpy as np
from contextlib import ExitStack

import concourse.bass as bass
import concourse.bacc as bacc
import concourse.tile as tile
from concourse import mybir
from concourse.bass_utils import run_bass_kernel_spmd

F32 = mybir.dt.float32
BF16 = mybir.dt.bfloat16

B, NA, NC, D = 512, 16, 256, 256
N_CORES = 8
G_CORE = B // N_CORES          # 64 graphs per core
G_CHUNK = 8                    # graphs per chunk (2 sub-chunks of 4)
SCALE = 1.0 / math.sqrt(D)

_CACHE = {}
LAST_EXEC_NS = None


def build_nc(g_core=G_CORE, dbg=False):
    chunks = g_core // G_CHUNK
    HA_ALL, HC_ALL = g_core * NA, g_core * NC

    nc = bacc.Bacc(None, target_bir_lowering=False, debug=False)
    h_ads = nc.declare_dram_parameter("h_ads", [HA_ALL, D], F32, isOutput=False)
    h_cat = nc.declare_dram_parameter("h_cat", [HC_ALL, D], F32, isOutput=False)
    h_cat_bf = nc.declare_dram_parameter("h_cat_bf", [HC_ALL, D], BF16, isOutput=False)
    h_ads_bf = nc.declare_dram_parameter("h_ads_bf", [HA_ALL, D], BF16, isOutput=False)
    wnames = ["wq_ads", "wk_ads", "wv_ads", "wq_cat", "wk_cat", "wv_cat"]
    bnames = ["bq_ads", "bk_ads", "bv_ads", "bq_cat", "bk_cat", "bv_cat"]
    W = {w: nc.declare_dram_parameter(w, [D, D], F32, isOutput=False) for w in wnames}
    Bv = {b: nc.declare_dram_parameter(b, [D], F32, isOutput=False) for b in bnames}
    ident_d = nc.declare_dram_parameter("ident", [128, 128], F32, isOutput=False)
    out = nc.declare_dram_parameter("out", [HA_ALL + HC_ALL, D], F32, isOutput=True)
    dmp = {}
    if dbg:
        for nm, shp in [("d_haT", [128, 2, 256]), ("d_qaT", [128, 2, 256]),
                        ("d_hcT", [128, 256]), ("d_s1", [128, 256]),
                        ("d_E1n", [128, 256]), ("d_u1", [128, 256]),
                        ("d_s2", [128, 256]), ("d_u2", [128, 257]),
                        ("d_va", [128, 257]), ("d_vc", [128, 256])]:
            dmp[nm] = nc.declare_dram_parameter(nm, shp, F32, isOutput=True)
    out_ads = out[0:HA_ALL, :]
    out_cat = out[HA_ALL:, :]

    with tile.TileContext(nc) as tc, ExitStack() as ctx:
        const = ctx.enter_context(tc.tile_pool(name="const", bufs=1))
        stage = ctx.enter_context(tc.tile_pool(name="stage", bufs=2))
        big = ctx.enter_context(tc.tile_pool(name="big", bufs=2))
        med = ctx.enter_context(tc.tile_pool(name="med", bufs=2))
        small = ctx.enter_context(tc.tile_pool(name="small", bufs=3))
        ps_pj = ctx.enter_context(tc.tile_pool(name="ps_pj", bufs=2, space="PSUM"))
        ps_a = ctx.enter_context(tc.tile_pool(name="ps_a", bufs=3, space="PSUM"))
        ps_d = ctx.enter_context(tc.tile_pool(name="ps_d", bufs=3, space="PSUM"))

        # ---- constants ----
        # weights -> bf16 [128(k%128), 2(kt), 256(j)]
        wstage = const.tile([128, 6, 2, 256], F32, tag="wstage")
        wbf = {}
        for i, w in enumerate(wnames):
            nc.gpsimd.dma_start(
                out=wstage[:, i, :, :],
                in_=W[w][:, :].rearrange("(t p) j -> p t j", p=128),
            )
            wb = const.tile([128, 2, 256], BF16, tag="w_" + w)
            nc.vector.tensor_copy(wb, wstage[:, i, :, :])
            wbf[w] = wb

        # per-partition bias tiles [128, 2] for transposed-layout outputs
        bcol = {}
        for b in ["bq_ads", "bk_ads", "bq_cat", "bk_cat"]:
            t = const.tile([128, 2], F32, tag="bc_" + b)
            for m in range(2):
                nc.gpsimd.dma_start(
                    out=t[:, m : m + 1],
                    in_=Bv[b][m * 128 : (m + 1) * 128].rearrange("(p o) -> p o", o=1),
                )
            bcol[b] = t

        # broadcast bias tiles [128, 256] (bias along free dim)
        bbc = {}
        for b in ["bv_ads", "bv_cat"]:
            t = const.tile([128, 256], F32, tag="bb_" + b)
            src = Bv[b][:]
            bc_ap = bass.AP(tensor=src.tensor, offset=src.offset, ap=[[0, 128]] + list(src.ap))
            nc.gpsimd.dma_start(out=t, in_=bc_ap)
            bbc[b] = t

        def dump(name, ap):
            if not dbg:
                return
            ft = med.tile(list(ap.shape), F32, tag="dump_" + name)
            nc.vector.tensor_copy(ft, ap)
            nc.sync.dma_start(out=dmp[name][...][tuple(slice(0, s) for s in ap.shape)], in_=ft)

        for ch in range(chunks):
            # ---------- loads ----------
            hc = big.tile([128, 16, 256], F32, tag="hc")  # rows t*128+p of chunk
            nc.sync.dma_start(
                out=hc,
                in_=h_cat[ch * 2048 : (ch + 1) * 2048, :].rearrange(
                    "(t p) d -> p t d", p=128
                ),
            )
            hcb = big.tile([128, 16, 256], BF16, tag="hcb")
            nc.sync.dma_start(
                out=hcb,
                in_=h_cat_bf[ch * 2048 : (ch + 1) * 2048, :].rearrange(
                    "(t p) d -> p t d", p=128
                ),
            )
            ha = med.tile([128, 2, 256], F32, tag="ha")  # padded: part 32q+a, a<16
            for t in range(2):
                for q in range(4):
                    r0 = ch * 128 + t * 64 + q * 16
                    nc.sync.dma_start(
                        out=ha[32 * q : 32 * q + 16, t, :],
                        in_=h_ads[r0 : r0 + 16, :],
                    )

            # ---------- transposes: hcT via DMA xbar, haT via PE ----------
            hcT = big.tile([128, 2, 2048], BF16, tag="hcT")  # [k%128, kt, n]
            for kb in range(2):
                nc.sync.dma_start(
                    out=hcT[:, kb, :],
                    in_=h_cat_bf[ch * 2048 : (ch + 1) * 2048, kb * 128 : (kb + 1) * 128],
                    transpose=True,
                )
            haT = med.tile([128, 2, 128], BF16, tag="haT")  # [k%128, kt, row(g*16+a)]
            for kb in range(2):
                nc.sync.dma_start(
                    out=haT[:, kb, :],
                    in_=h_ads_bf[ch * 128 : (ch + 1) * 128, kb * 128 : (kb + 1) * 128],
                    transpose=True,
                )

            if ch == 0:
                dump("d_haT", haT)
                dump("d_hcT", hcT[:, 0, 0:256])
            # fold output-side V biases into residual inputs (after transposes!)
            for t in range(16):
                nc.gpsimd.tensor_add(hc[:, t, :], hc[:, t, :], bbc["bv_ads"])
            for t in range(2):
                nc.gpsimd.tensor_add(ha[:, t, :], ha[:, t, :], bbc["bv_cat"])

            # ---------- projections (cat) ----------
            qcT = big.tile([128, 2, 2048], BF16, tag="qcT")
            kcT = big.tile([128, 2, 2048], BF16, tag="kcT")
            for wname, dst, bias in (
                ("wq_cat", qcT, bcol["bq_cat"]),
                ("wk_cat", kcT, bcol["bk_cat"]),
            ):
                for m in range(2):
                    for nchk in range(4):
                        pp = ps_pj.tile([128, 512], F32, tag="pp")
                        for kt in range(2):
                            nc.tensor.matmul(
                                pp,
                                wbf[wname][:, kt, m * 128 : (m + 1) * 128],
                                hcT[:, kt, nchk * 512 : (nchk + 1) * 512],
                                start=(kt == 0),
                                stop=(kt == 1),
                            )
                        nc.vector.tensor_scalar_add(
                            dst[:, m, nchk * 512 : (nchk + 1) * 512], pp, bias[:, m : m + 1]
                        )

            # ---------- projections (ads) ----------
            qaT = med.tile([128, 2, 128], BF16, tag="qaT")
            kaT = med.tile([128, 2, 128], BF16, tag="kaT")
            for wname, dst, bias in (
                ("wq_ads", qaT, bcol["bq_ads"]),
                ("wk_ads", kaT, bcol["bk_ads"]),
            ):
                for m in range(2):
                    pp = ps_pj.tile([128, 512], F32, tag="pp")
                    for kt in range(2):
                        nc.tensor.matmul(
                            pp[:, 0:128],
                            wbf[wname][:, kt, m * 128 : (m + 1) * 128],
                            haT[:, kt, :],
                            start=(kt == 0),
                            stop=(kt == 1),
                        )
                    nc.vector.tensor_scalar_add(dst[:, m, :], pp[:, 0:128], bias[:, m : m + 1])
            # va padded natural [32q+a, t, d] + ones column (col 256), NO bias
            va = med.tile([128, 2, 257], BF16, tag="va")
            for t in range(2):
                pv = ps_a.tile([128, 257], F32, tag="aa")
                for q in range(4):
                    g = t * 4 + q
                    for kt in range(2):
                        nc.tensor.matmul(
                            pv[32 * q : 32 * q + 16, 0:256],
                            haT[:, kt, g * 16 : g * 16 + 16],
                            wbf["wv_ads"][:, kt, :],
                            start=(kt == 0),
                            stop=(kt == 1),
                            tile_position=(0, 32 * q),
                        )
                nc.scalar.copy(va[:, t, 0:256], pv[:, 0:256])
                if ch < 2:
                    nc.vector.memset(va[:, t, 256:257], 1.0)

            if ch == 0:
                dump("d_qaT", qaT)
                dump("d_va", va[:, 0, :])
            # ---------- ads-side attention ----------
            E1n = med.tile([128, 2, 256], BF16, tag="E1n")
            r1 = small.tile([128, 2], F32, tag="r1")
            r1i = small.tile([128, 2], F32, tag="r1i")
            E1f = med.tile([128, 2, 256], F32, tag="E1f")
            for t in range(2):
                s1 = ps_a.tile([128, 256], F32, tag="aa")
                for q in range(4):
                    g = t * 4 + q
                    for jm in range(2):
                        nc.tensor.matmul(
                            s1[32 * q : 32 * q + 16, :],
                            qaT[:, jm, g * 16 : g * 16 + 16],
                            kcT[:, jm, g * 256 : (g + 1) * 256],
                            start=(jm == 0),
                            stop=(jm == 1),
                            tile_position=(0, 32 * q),
                        )
                if ch == 0 and t == 0:
                    dump("d_s1", s1)
                nc.scalar.activation(
                    out=E1f[:, t, :],
                    in_=s1,
                    func=mybir.ActivationFunctionType.Exp,
                    accum_out=r1[:, t : t + 1],
                )
            nc.vector.reciprocal(r1i, r1)
            for t in range(2):
                nc.vector.tensor_scalar_mul(E1n[:, t, :], E1f[:, t, :], r1i[:, t : t + 1])
            if ch == 0:
                dump("d_E1n", E1n[:, 0, :])
            # E1nT: [128(c%128), cb, t*128 + padded_a]
            E1nT = med.tile([128, 2, 256], BF16, tag="E1nT")
            for t in range(2):
                for cb in range(2):
                    nc.sync.dma_start(
                        out=E1nT[:, cb, t * 128 : (t + 1) * 128],
                        in_=E1n[:, t, cb * 128 : (cb + 1) * 128],
                        transpose=True,
                    )
            # AV1 + epilogue (ads)
            y_ads = med.tile([128, 2, 256], F32, tag="y_ads")
            ss_a = small.tile([128, 2], F32, tag="ss_a")
            sn_a = small.tile([128, 2], F32, tag="sn_a")
            sq_scr = med.tile([128, 256], F32, tag="sq_scr")
            T1sb = med.tile([128, 2, 256], BF16, tag="T1sb")
            for t in range(2):
                t1 = ps_a.tile([128, 256], F32, tag="aa")
                for q in range(4):
                    g = t * 4 + q
                    for cb in range(2):
                        nc.tensor.matmul(
                            t1[32 * q : 32 * q + 16, :],
                            E1nT[:, cb, t * 128 + 32 * q : t * 128 + 32 * q + 16],
                            hcb[:, 2 * g + cb, :],
                            start=(cb == 0),
                            stop=(cb == 1),
                            tile_position=(0, 32 * q),
                        )
                nc.scalar.copy(T1sb[:, t, :], t1)
            T1T = med.tile([128, 2, 256], BF16, tag="T1T")  # [k%128, kt, t*128+pad_a]
            for t in range(2):
                for kt in range(2):
                    nc.sync.dma_start(
                        out=T1T[:, kt, t * 128 : (t + 1) * 128],
                        in_=T1sb[:, t, kt * 128 : (kt + 1) * 128],
                        transpose=True,
                    )
            for t in range(2):
                u1 = ps_d.tile([128, 256], F32, tag="dd")
                for kt in range(2):
                    nc.tensor.matmul(
                        u1,
                        T1T[:, kt, t * 128 : (t + 1) * 128],
                        wbf["wv_cat"][:, kt, :],
                        start=(kt == 0),
                        stop=(kt == 1),
                    )
                if ch == 0 and t == 0:
                    dump("d_u1", u1)
                nc.vector.tensor_add(y_ads[:, t, :], u1, ha[:, t, :])
                nc.scalar.activation(
                    out=sq_scr,
                    in_=y_ads[:, t, :],
                    func=mybir.ActivationFunctionType.Square,
                    accum_out=ss_a[:, t : t + 1],
                )

            nc.scalar.sqrt(sn_a, ss_a)
            nc.vector.reciprocal(sn_a, sn_a)
            o_ads = med.tile([128, 2, 256], F32, tag="o_ads")
            for t in range(2):
                nc.scalar.activation(
                    out=o_ads[:, t, :],
                    in_=y_ads[:, t, :],
                    func=mybir.ActivationFunctionType.Copy,
                    scale=sn_a[:, t : t + 1],
                )
                for q in range(4):
                    r0 = ch * 128 + t * 64 + q * 16
                    nc.sync.dma_start(
                        out=out_ads[r0 : r0 + 16, :],
                        in_=o_ads[32 * q : 32 * q + 16, t, :],
                    )

            # ---------- cat-side attention ----------
            E2 = med.tile([128, 2, 256], BF16, tag="E2")  # padded [32q+a, t, c]
            for t in range(2):
                s2 = ps_a.tile([128, 256], F32, tag="aa")
                for q in range(4):
                    g = t * 4 + q
                    for jm in range(2):
                        nc.tensor.matmul(
                            s2[32 * q : 32 * q + 16, :],
                            kaT[:, jm, g * 16 : g * 16 + 16],
                            qcT[:, jm, g * 256 : (g + 1) * 256],
                            start=(jm == 0),
                            stop=(jm == 1),
                            tile_position=(0, 32 * q),
                        )
                if ch == 0 and t == 0:
                    dump("d_s2", s2)
                nc.scalar.activation(
                    out=E2[:, t, :], in_=s2, func=mybir.ActivationFunctionType.Exp
                )
            # AV2 (per graph, row-tiled) + epilogue (cat), batched norm
            y_all = big.tile([128, 16, 256], F32, tag="y_all")
            ss_c = small.tile([128, 16], F32, tag="ss_c")
            sn_c = small.tile([128, 16], F32, tag="sn_c")
            scr = med.tile([128, 256], F32, tag="sq_scr2")
            for t in range(2):
                for q in range(4):
                    g = t * 4 + q
                    for cb in range(2):
                        idx = 2 * g + cb
                        u2 = ps_d.tile([128, 257], F32, tag="dd")
                        nc.tensor.matmul(
                            u2,
                            E2[32 * q : 32 * q + 16, t, cb * 128 : (cb + 1) * 128],
                            va[32 * q : 32 * q + 16, t, :],
                            start=True,
                            stop=True,
                            tile_position=(32 * q, 0),
                        )
                        if ch == 0 and t == 0 and q == 0 and cb == 0:
                            dump("d_u2", u2)
                        r2i = small.tile([128, 1], F32, tag="r2i")
                        nc.vector.reciprocal(r2i, u2[:, 256:257])
                        y = y_all[:, idx, :]
                        nc.vector.tensor_scalar_mul(y, u2[:, 0:256], r2i)
                        nc.gpsimd.tensor_add(y, y, hc[:, idx, :])
                        nc.scalar.activation(
                            out=scr,
                            in_=y,
                            func=mybir.ActivationFunctionType.Square,
                            accum_out=ss_c[:, idx : idx + 1],
                        )
            nc.scalar.sqrt(sn_c, ss_c)
            nc.vector.reciprocal(sn_c, sn_c)
            for idx in range(16):
                oc = med.tile([128, 256], F32, tag="o_cat")
                if idx % 2 == 0:
                    nc.scalar.activation(
                        out=oc,
                        in_=y_all[:, idx, :],
                        func=mybir.ActivationFunctionType.Copy,
                        scale=sn_c[:, idx : idx + 1],
                    )
                else:
                    nc.vector.tensor_scalar_mul(oc, y_all[:, idx, :], sn_c[:, idx : idx + 1])
                nc.sync.dma_start(
                    out=out_cat[ch * 2048 + idx * 128 : ch * 2048 + (idx + 1) * 128, :],
                    in_=oc,
                )
    nc.compile()
    return nc


def _prep_host(inputs):
    f = lambda x: np.ascontiguousarray(np.asarray(x, dtype=np.float32))
    wm = {}
    wm["wq_ads"] = f(np.asarray(inputs["Wq_ads"]).T * SCALE)
    wm["bq_ads"] = f(np.asarray(inputs["bq_ads"]) * SCALE)
    wm["wk_ads"] = f(np.asarray(inputs["Wk_ads"]).T)
    wm["bk_ads"] = f(inputs["bk_ads"])
    wm["wv_ads"] = f(np.asarray(inputs["Wv_ads"]).T)
    wm["bv_ads"] = f(inputs["bv_ads"])
    wm["wq_cat"] = f(np.asarray(inputs["Wq_cat"]).T * SCALE)
    wm["bq_cat"] = f(np.asarray(inputs["bq_cat"]) * SCALE)
    wm["wk_cat"] = f(np.asarray(inputs["Wk_cat"]).T)
    wm["bk_cat"] = f(inputs["bk_cat"])
    wm["wv_cat"] = f(np.asarray(inputs["Wv_cat"]).T)
    wm["bv_cat"] = f(inputs["bv_cat"])
    wm["ident"] = np.eye(128, dtype=np.float32)
    return wm


DUMPS = None


def _install_profile_hook():
    """Provide antenv.axon_hooks + the ctypes NTFF hook this container lacks."""
    import sys, types, ctypes, contextlib

    if "antenv.axon_hooks" in sys.modules:
        return
    mod = types.ModuleType("antenv.axon_hooks")
    state = {"hook": None}
    mod.set_axon_ntff_profile_hook = lambda h: state.__setitem__("hook", h)
    mod.get_axon_ntff_profile_hook = lambda: state["hook"]
    sys.modules["antenv.axon_hooks"] = mod
    import antenv
    antenv.axon_hooks = mod

    lib = ctypes.CDLL("/opt/axon/libaxon_pjrt.so")
    if not hasattr(lib, "axon_start_nrt_profile"):
        return
    lib.axon_start_nrt_profile.argtypes = [ctypes.POINTER(ctypes.c_int64), ctypes.c_size_t]
    lib.axon_start_nrt_profile.restype = ctypes.c_int64
    lib.axon_stop_nrt_profile.argtypes = [ctypes.c_char_p]
    lib.axon_stop_nrt_profile.restype = ctypes.c_int64

    @contextlib.contextmanager
    def _hook(output_dir, device_ids):
        import jax
        jax.devices()
        if device_ids:
            ids = (ctypes.c_int64 * len(device_ids))(*device_ids)
            rc = lib.axon_start_nrt_profile(ids, len(device_ids))
        else:
            rc = lib.axon_start_nrt_profile(None, 0)
        if rc != 0:
            raise RuntimeError(f"axon_start_nrt_profile rc={rc}")
        try:
            yield
        finally:
            n = lib.axon_stop_nrt_profile(str(output_dir).encode())
            print(f"profile: {n} file(s) written to {output_dir}")

    mod.set_axon_ntff_profile_hook(_hook)
    # avoid bucket uploads from this container
    import concourse.bass_utils as _bu
    _bu.upload_artifacts = lambda d: str(d)


def kernel(trace=False, g_core=G_CORE, dbg=False, **inputs):
    global LAST_EXEC_NS, DUMPS
    key = (g_core, dbg)
    if key not in _CACHE:
        _CACHE[key] = build_nc(g_core, dbg)
    nc = _CACHE[key]
    if trace:
        _install_profile_hook()

    H_ads = np.ascontiguousarray(np.asarray(inputs["h_ads"], dtype=np.float32))
    H_cat = np.ascontiguousarray(np.asarray(inputs["h_cat"], dtype=np.float32))
    wm = _prep_host(inputs)

    na_core, nc_core = g_core * NA, g_core * NC
    in_maps = []
    for c in range(N_CORES):
        m = dict(wm)
        m["h_ads"] = H_ads[c * na_core : (c + 1) * na_core]
        m["h_cat"] = H_cat[c * nc_core : (c + 1) * nc_core]
        m["h_cat_bf"] = np.ascontiguousarray(m["h_cat"].astype(ml_dtypes.bfloat16))
        m["h_ads_bf"] = np.ascontiguousarray(m["h_ads"].astype(ml_dtypes.bfloat16))
        in_maps.append(m)

    res = run_bass_kernel_spmd(nc, in_maps, list(range(N_CORES)), trace=trace)
    LAST_EXEC_NS = getattr(res, "exec_time_ns", None)
    outs = res.results
    DUMPS = outs
    new_h_ads = np.concatenate([outs[c]["out"][:na_core] for c in range(N_CORES)], axis=0)
    new_h_cat = np.concatenate([outs[c]["out"][na_core:] for c in range(N_CORES)], axis=0)
    return new_h_ads, new_h_cat


def bench(iters=8, g_core=G_CORE, **inputs):
    """Run once for outputs + time repeated executions of the compiled NEFF."""
    global LAST_EXEC_NS
    import time
    import jax
    from jax.sharding import Mesh, PartitionSpec
    from jax.experimental.shard_map import shard_map
    from concourse import bass2jax, mybir as _mb

    key = (g_core, False)
    if key not in _CACHE:
        _CACHE[key] = build_nc(g_core, False)
    nc = _CACHE[key]
    bass2jax.install_neuronx_cc_hook()

    H_ads = np.ascontiguousarray(np.asarray(inputs["h_ads"], dtype=np.float32))
    H_cat = np.ascontiguousarray(np.asarray(inputs["h_cat"], dtype=np.float32))
    wm = _prep_host(inputs)
    na_core, nc_core = g_core * NA, g_core * NC
    in_maps = []
    for c in range(N_CORES):
        m = dict(wm)
        m["h_ads"] = H_ads[c * na_core : (c + 1) * na_core]
        m["h_cat"] = H_cat[c * nc_core : (c + 1) * nc_core]
        m["h_cat_bf"] = np.ascontiguousarray(m["h_cat"].astype(ml_dtypes.bfloat16))
        m["h_ads_bf"] = np.ascontiguousarray(m["h_ads"].astype(ml_dtypes.bfloat16))
        in_maps.append(m)

    partition_name = nc.partition_id_tensor.name if nc.partition_id_tensor else None
    in_names, out_names, out_avals, zero_outs = [], [], [], []
    for alloc in nc.m.functions[0].allocations:
        if not isinstance(alloc, _mb.MemoryLocationSet):
            continue
        name = alloc.memorylocations[0].name
        if alloc.kind == "ExternalInput":
            if name != partition_name:
                in_names.append(name)
        elif alloc.kind == "ExternalOutput":
            shape = tuple(alloc.tensor_shape)
            dtype = _mb.dt.np(alloc.dtype)
            out_avals.append(jax.core.ShapedArray(shape, dtype))
            out_names.append(name)
            zero_outs.append(np.zeros(shape, dtype))
    n_params, n_outs = len(in_names), len(out_avals)
    in_names_all = in_names + out_names + ([partition_name] if partition_name else [])

    def _body(*args):
        operands = list(args)
        if partition_name is not None:
            operands.append(bass2jax.partition_id_tensor())
        outs = bass2jax._bass_exec_p.bind(
            *operands,
            out_avals=tuple(out_avals),
            in_names=tuple(in_names_all),
            out_names=tuple(out_names),
            lowering_input_output_aliases=(),
            sim_require_finite=True,
            sim_require_nnan=True,
            nc=nc,
        )
        return tuple(outs)

    devices = jax.devices()[:N_CORES]
    mesh = Mesh(np.asarray(devices), ("core",))
    donate = tuple(range(n_params, n_params + n_outs))
    sharded = jax.jit(
        shard_map(_body, mesh=mesh,
                  in_specs=(PartitionSpec("core"),) * (n_params + n_outs),
                  out_specs=(PartitionSpec("core"),) * n_outs, check_rep=False),
        keep_unused=True,
    )
    concat_in = [np.concatenate([np.asarray(in_maps[c][k]) for c in range(N_CORES)], axis=0)
                 for k in in_names]
    concat_zero = [np.zeros((N_CORES * z.shape[0], *z.shape[1:]), z.dtype) for z in zero_outs]
    from jax.sharding import NamedSharding
    shard = NamedSharding(mesh, PartitionSpec("core"))
    dev_in = [jax.device_put(a, shard) for a in concat_in]

    # warmup/compile + correctness outputs
    out_arrs = sharded(*dev_in, *[jax.device_put(z, shard) for z in concat_zero])
    del donate
    jax.block_until_ready(out_arrs)
    out0 = np.asarray(out_arrs[0]).reshape(N_CORES, -1, D)

    zs = [jax.device_put(z, shard) for z in concat_zero]
    jax.block_until_ready(zs)
    times = []
    for _ in range(iters):
        t0 = time.perf_counter()
        o = sharded(*dev_in, *zs)
        jax.block_until_ready(o)
        times.append(time.perf_counter() - t0)
    LAST_EXEC_NS = int(min(times) * 1e9)

    new_h_ads = np.concatenate([out0[c][:na_core] for c in range(N_CORES)], axis=0)
    new_h_cat = np.concatenate([out0[c][na_core:] for c in range(N_CORES)], axis=0)
    return new_h_ads, new_h_cat, LAST_EXEC_NS, times


# revision 21
# speedup vs baseline: 1.7487x; 1.7487x over previous
"""Trainium2 Bass kernel for AttentionInteraction (cross-attention between
ads/cat node blocks of B=512 graphs, data-parallel over 8 NeuronCores).

Self-contained: hardcodes shapes B=512, NA=16, NC=256, D=256, 8 cores.
kernel(**inputs) takes the FULL unsharded inputs and returns
(new_h_ads, new_h_cat) like the reference.
"""

import math
import ml_dtypes
import numpy as np
from contextlib import ExitStack

import concourse.bass as bass
import concourse.bacc as bacc
import concourse.tile as tile
from concourse import mybir
from concourse.bass_utils import run_bass_kernel_spmd

F32 = mybir.dt.float32
BF16 = mybir.dt.bfloat16

B, NA, NC, D = 512, 16, 256, 256
N_CORES = 8
G_CORE = B // N_CORES          # 64 graphs per core
G_CHUNK = 8                    # graphs per chunk (2 sub-chunks of 4)
SCALE = 1.0 / math.sqrt(D)

_CACHE = {}
LAST_EXEC_NS = None


def build_nc(g_core=G_CORE, dbg=False):
    chunks = g_core // G_CHUNK
    HA_ALL, HC_ALL = g_core * NA, g_core * NC

    nc = bacc.Bacc(None, target_bir_lowering=False, debug=False)
    h_ads = nc.declare_dram_parameter("h_ads", [HA_ALL, D], F32, isOutput=False)
    h_cat = nc.declare_dram_parameter("h_cat", [HC_ALL, D], F32, isOutput=False)
    h_cat_bf = nc.declare_dram_parameter("h_cat_bf", [HC_ALL, D], BF16, isOutput=False)
    h_ads_bf = nc.declare_dram_parameter("h_ads_bf", [HA_ALL, D], BF16, isOutput=False)
    wnames = ["wq_ads", "wk_ads", "wv_ads", "wq_cat", "wk_cat", "wv_cat"]
    bnames = ["bq_ads", "bk_ads", "bv_ads", "bq_cat", "bk_cat", "bv_cat"]
    W = {w: nc.declare_dram_parameter(w, [D, D], F32, isOutput=False) for w in wnames}
    Bv = {b: nc.declare_dram_parameter(b, [D], F32, isOutput=False) for b in bnames}
    ident_d = nc.declare_dram_parameter("ident", [128, 128], F32, isOutput=False)
    out = nc.declare_dram_parameter("out", [HA_ALL + HC_ALL, D], F32, isOutput=True)
    dmp = {}
    if dbg:
        for nm, shp in [("d_haT", [128, 2, 256]), ("d_qaT", [128, 2, 256]),
                        ("d_hcT", [128, 256]), ("d_s1", [128, 256]),
                        ("d_E1n", [128, 256]), ("d_u1", [128, 256]),
                        ("d_s2", [128, 256]), ("d_u2", [128, 257]),
                        ("d_va", [128, 257]), ("d_vc", [128, 256])]:
            dmp[nm] = nc.declare_dram_parameter(nm, shp, F32, isOutput=True)
    out_ads = out[0:HA_ALL, :]
    out_cat = out[HA_ALL:, :]

    with tile.TileContext(nc) as tc, ExitStack() as ctx:
        const = ctx.enter_context(tc.tile_pool(name="const", bufs=1))
        stage = ctx.enter_context(tc.tile_pool(name="stage", bufs=2))
        big = ctx.enter_context(tc.tile_pool(name="big", bufs=2))
        med = ctx.enter_context(tc.tile_pool(name="med", bufs=2))
        small = ctx.enter_context(tc.tile_pool(name="small", bufs=3))
        ps_pj = ctx.enter_context(tc.tile_pool(name="ps_pj", bufs=2, space="PSUM"))
        ps_a = ctx.enter_context(tc.tile_pool(name="ps_a", bufs=3, space="PSUM"))
        ps_d = ctx.enter_context(tc.tile_pool(name="ps_d", bufs=3, space="PSUM"))

        # ---- constants ----
        identf = const.tile([128, 128], F32, tag="identf")
        nc.sync.dma_start(out=identf, in_=ident_d[:, :])
        ident16 = const.tile([128, 128], BF16, tag="ident16")
        nc.vector.tensor_copy(ident16, identf)

        # weights -> bf16 [128(k%128), 2(kt), 256(j)]
        wstage = const.tile([128, 6, 2, 256], F32, tag="wstage")
        wbf = {}
        for i, w in enumerate(wnames):
            nc.gpsimd.dma_start(
                out=wstage[:, i, :, :],
                in_=W[w][:, :].rearrange("(t p) j -> p t j", p=128),
            )
            wb = const.tile([128, 2, 256], BF16, tag="w_" + w)
            nc.vector.tensor_copy(wb, wstage[:, i, :, :])
            wbf[w] = wb

        # per-partition bias tiles [128, 2] for transposed-layout outputs
        bcol = {}
        for b in ["bq_ads", "bk_ads", "bq_cat", "bk_cat"]:
            t = const.tile([128, 2], F32, tag="bc_" + b)
            for m in range(2):
                nc.gpsimd.dma_start(
                    out=t[:, m : m + 1],
                    in_=Bv[b][m * 128 : (m + 1) * 128].rearrange("(p o) -> p o", o=1),
                )
            bcol[b] = t

        # broadcast bias tiles [128, 256] (bias along free dim)
        bbc = {}
        for b in ["bv_ads", "bv_cat"]:
            t = const.tile([128, 256], F32, tag="bb_" + b)
            src = Bv[b][:]
            bc_ap = bass.AP(tensor=src.tensor, offset=src.offset, ap=[[0, 128]] + list(src.ap))
            nc.gpsimd.dma_start(out=t, in_=bc_ap)
            bbc[b] = t

        def dump(name, ap):
            if not dbg:
                return
            ft = med.tile(list(ap.shape), F32, tag="dump_" + name)
            nc.vector.tensor_copy(ft, ap)
            nc.sync.dma_start(out=dmp[name][...][tuple(slice(0, s) for s in ap.shape)], in_=ft)

        for ch in range(chunks):
            # ---------- loads ----------
            hcb = big.tile([128, 16, 256], BF16, tag="hcb")
            nc.sync.dma_start(
                out=hcb,
                in_=h_cat_bf[ch * 2048 : (ch + 1) * 2048, :].rearrange(
                    "(t p) d -> p t d", p=128
                ),
            )
            ha = med.tile([128, 2, 256], F32, tag="ha")  # padded: part 32q+a, a<16
            for t in range(2):
                for q in range(4):
                    r0 = ch * 128 + t * 64 + q * 16
                    nc.sync.dma_start(
                        out=ha[32 * q : 32 * q + 16, t, :],
                        in_=h_ads[r0 : r0 + 16, :],
                    )

            # ---------- transposes: hcT via DMA xbar, haT via PE ----------
            hcT = big.tile([128, 2, 2048], BF16, tag="hcT")  # [k%128, kt, n]
            for kb in range(2):
                nc.sync.dma_start(
                    out=hcT[:, kb, :],
                    in_=h_cat_bf[ch * 2048 : (ch + 1) * 2048, kb * 128 : (kb + 1) * 128],
                    transpose=True,
                )
            haT = med.tile([128, 2, 128], BF16, tag="haT")  # [k%128, kt, row(g*16+a)]
            for kb in range(2):
                nc.sync.dma_start(
                    out=haT[:, kb, :],
                    in_=h_ads_bf[ch * 128 : (ch + 1) * 128, kb * 128 : (kb + 1) * 128],
                    transpose=True,
                )

            if ch == 0:
                dump("d_haT", haT)
                dump("d_hcT", hcT[:, 0, 0:256])
            # fold bv_cat into ads residual (attention rows sum to 1)
            for t in range(2):
                nc.gpsimd.tensor_add(ha[:, t, :], ha[:, t, :], bbc["bv_cat"])

            # ---------- projections (cat) ----------
            qcT = big.tile([128, 2, 2048], BF16, tag="qcT")
            kcT = big.tile([128, 2, 2048], BF16, tag="kcT")
            for wname, dst, bias in (
                ("wq_cat", qcT, bcol["bq_cat"]),
                ("wk_cat", kcT, bcol["bk_cat"]),
            ):
                for m in range(2):
                    for nchk in range(4):
                        pp = ps_pj.tile([128, 512], F32, tag="pp")
                        for kt in range(2):
                            nc.tensor.matmul(
                                pp,
                                wbf[wname][:, kt, m * 128 : (m + 1) * 128],
                                hcT[:, kt, nchk * 512 : (nchk + 1) * 512],
                                start=(kt == 0),
                                stop=(kt == 1),
                            )
                        nc.vector.tensor_scalar_add(
                            dst[:, m, nchk * 512 : (nchk + 1) * 512], pp, bias[:, m : m + 1]
                        )

            # ---------- projections (ads) ----------
            qaT = med.tile([128, 2, 128], BF16, tag="qaT")
            kaT = med.tile([128, 2, 128], BF16, tag="kaT")
            for wname, dst, bias in (
                ("wq_ads", qaT, bcol["bq_ads"]),
                ("wk_ads", kaT, bcol["bk_ads"]),
            ):
                for m in range(2):
                    pp = ps_pj.tile([128, 512], F32, tag="pp")
                    for kt in range(2):
                        nc.tensor.matmul(
                            pp[:, 0:128],
                            wbf[wname][:, kt, m * 128 : (m + 1) * 128],
                            haT[:, kt, :],
                            start=(kt == 0),
                            stop=(kt == 1),
                        )
                    nc.vector.tensor_scalar_add(dst[:, m, :], pp[:, 0:128], bias[:, m : m + 1])
            # va padded natural [32q+a, t, d] + ones column (col 256), NO bias
            va = med.tile([128, 2, 257], BF16, tag="va")
            for t in range(2):
                pv = ps_a.tile([128, 257], F32, tag="aa")
                for q in range(4):
                    g = t * 4 + q
                    for kt in range(2):
                        nc.tensor.matmul(
                            pv[32 * q : 32 * q + 16, 0:256],
                            haT[:, kt, g * 16 : g * 16 + 16],
                            wbf["wv_ads"][:, kt, :],
                            start=(kt == 0),
                            stop=(kt == 1),
                            tile_position=(0, 32 * q),
                        )
                nc.vector.tensor_add(va[:, t, 0:256], pv[:, 0:256], bbc["bv_ads"])
                if ch < 2:
                    nc.vector.memset(va[:, t, 256:257], 1.0)

            if ch == 0:
                dump("d_qaT", qaT)
                dump("d_va", va[:, 0, :])
            # ---------- ads-side attention ----------
            E1n = med.tile([128, 2, 256], BF16, tag="E1n")
            r1 = small.tile([128, 2], F32, tag="r1")
            r1i = small.tile([128, 2], F32, tag="r1i")
            E1f = med.tile([128, 2, 256], F32, tag="E1f")
            for t in range(2):
                s1 = ps_a.tile([128, 256], F32, tag="aa")
                for q in range(4):
                    g = t * 4 + q
                    for jm in range(2):
                        nc.tensor.matmul(
                            s1[32 * q : 32 * q + 16, :],
                            qaT[:, jm, g * 16 : g * 16 + 16],
                            kcT[:, jm, g * 256 : (g + 1) * 256],
                            start=(jm == 0),
                            stop=(jm == 1),
                            tile_position=(0, 32 * q),
                        )
                if ch == 0 and t == 0:
                    dump("d_s1", s1)
                nc.scalar.activation(
                    out=E1f[:, t, :],
                    in_=s1,
                    func=mybir.ActivationFunctionType.Exp,
                    accum_out=r1[:, t : t + 1],
                )
            nc.vector.reciprocal(r1i, r1)
            for t in range(2):
                nc.vector.tensor_scalar_mul(E1n[:, t, :], E1f[:, t, :], r1i[:, t : t + 1])
            if ch == 0:
                dump("d_E1n", E1n[:, 0, :])
            # E1nT: [128(c%128), cb, t*128 + padded_a]
            E1nT = med.tile([128, 2, 256], BF16, tag="E1nT")
            for t in range(2):
                for cb in range(2):
                    pt = ps_a.tile([128, 128], BF16, tag="aa")
                    nc.tensor.transpose(pt, E1n[:, t, cb * 128 : (cb + 1) * 128], ident16)
                    nc.scalar.copy(E1nT[:, cb, t * 128 : (t + 1) * 128], pt)
            # AV1 + epilogue (ads)
            y_ads = med.tile([128, 2, 256], F32, tag="y_ads")
            ss_a = small.tile([128, 2], F32, tag="ss_a")
            sn_a = small.tile([128, 2], F32, tag="sn_a")
            sq_scr = med.tile([128, 256], F32, tag="sq_scr")
            T1sb = med.tile([128, 2, 256], BF16, tag="T1sb")
            for t in range(2):
                t1 = ps_a.tile([128, 256], F32, tag="aa")
                for q in range(4):
                    g = t * 4 + q
                    for cb in range(2):
                        nc.tensor.matmul(
                            t1[32 * q : 32 * q + 16, :],
                            E1nT[:, cb, t * 128 + 32 * q : t * 128 + 32 * q + 16],
                            hcb[:, 2 * g + cb, :],
                            start=(cb == 0),
                            stop=(cb == 1),
                            tile_position=(0, 32 * q),
                        )
                nc.scalar.copy(T1sb[:, t, :], t1)
            T1T = med.tile([128, 2, 256], BF16, tag="T1T")  # [k%128, kt, t*128+pad_a]
            for t in range(2):
                for kt in range(2):
                    pt = ps_a.tile([128, 128], BF16, tag="aa")
                    nc.tensor.transpose(pt, T1sb[:, t, kt * 128 : (kt + 1) * 128], ident16)
                    nc.scalar.copy(T1T[:, kt, t * 128 : (t + 1) * 128], pt)
            for t in range(2):
                u1 = ps_d.tile([128, 256], F32, tag="dd")
                for kt in range(2):
                    nc.tensor.matmul(
                        u1,
                        T1T[:, kt, t * 128 : (t + 1) * 128],
                        wbf["wv_cat"][:, kt, :],
                        start=(kt == 0),
                        stop=(kt == 1),
                    )
                if ch == 0 and t == 0:
                    dump("d_u1", u1)
                nc.vector.tensor_add(y_ads[:, t, :], u1, ha[:, t, :])
                nc.scalar.activation(
                    out=sq_scr,
                    in_=y_ads[:, t, :],
                    func=mybir.ActivationFunctionType.Square,
                    accum_out=ss_a[:, t : t + 1],
                )

            nc.scalar.sqrt(sn_a, ss_a)
            nc.vector.reciprocal(sn_a, sn_a)
            o_ads = med.tile([128, 2, 256], F32, tag="o_ads")
            for t in range(2):
                nc.scalar.activation(
                    out=o_ads[:, t, :],
                    in_=y_ads[:, t, :],
                    func=mybir.ActivationFunctionType.Copy,
                    scale=sn_a[:, t : t + 1],
                )
                for q in range(4):
                    r0 = ch * 128 + t * 64 + q * 16
                    nc.sync.dma_start(
                        out=out_ads[r0 : r0 + 16, :],
                        in_=o_ads[32 * q : 32 * q + 16, t, :],
                    )

            # ---------- cat-side attention ----------
            E2 = med.tile([128, 2, 256], BF16, tag="E2")  # padded [32q+a, t, c]
            for t in range(2):
                s2 = ps_a.tile([128, 256], F32, tag="aa")
                for q in range(4):
                    g = t * 4 + q
                    for jm in range(2):
                        nc.tensor.matmul(
                            s2[32 * q : 32 * q + 16, :],
                            kaT[:, jm, g * 16 : g * 16 + 16],
                            qcT[:, jm, g * 256 : (g + 1) * 256],
                            start=(jm == 0),
                            stop=(jm == 1),
                            tile_position=(0, 32 * q),
                        )
                if ch == 0 and t == 0:
                    dump("d_s2", s2)
                nc.scalar.activation(
                    out=E2[:, t, :], in_=s2, func=mybir.ActivationFunctionType.Exp
                )
            # AV2 (per graph, row-tiled) + epilogue (cat), batched norm
            y_all = big.tile([128, 16, 256], F32, tag="y_all")
            ss_c = small.tile([128, 16], F32, tag="ss_c")
            sn_c = small.tile([128, 16], F32, tag="sn_c")
            scr = med.tile([128, 256], F32, tag="sq_scr2")
            for t in range(2):
                for q in range(4):
                    g = t * 4 + q
                    for cb in range(2):
                        idx = 2 * g + cb
                        u2 = ps_d.tile([128, 257], F32, tag="dd")
                        nc.tensor.matmul(
                            u2,
                            E2[32 * q : 32 * q + 16, t, cb * 128 : (cb + 1) * 128],
                            va[32 * q : 32 * q + 16, t, :],
                            start=True,
                            stop=True,
                            tile_position=(32 * q, 0),
                        )
                        if ch == 0 and t == 0 and q == 0 and cb == 0:
                            dump("d_u2", u2)
                        r2i = small.tile([128, 1], F32, tag="r2i")
                        nc.vector.reciprocal(r2i, u2[:, 256:257])
                        y = y_all[:, idx, :]
                        nc.vector.tensor_scalar_mul(y, u2[:, 0:256], r2i)
                        nc.gpsimd.tensor_add(y, y, hcb[:, idx, :])
                        nc.scalar.activation(
                            out=scr,
                            in_=y,
                            func=mybir.ActivationFunctionType.Square,
                            accum_out=ss_c[:, idx : idx + 1],
                        )
            nc.scalar.sqrt(sn_c, ss_c)
            nc.vector.reciprocal(sn_c, sn_c)
            for idx in range(16):
                if idx % 2 == 0:
                    nc.scalar.activation(
                        out=y_all[:, idx, :],
                        in_=y_all[:, idx, :],
                        func=mybir.ActivationFunctionType.Copy,
                        scale=sn_c[:, idx : idx + 1],
                    )
                else:
                    nc.vector.tensor_scalar_mul(
                        y_all[:, idx, :], y_all[:, idx, :], sn_c[:, idx : idx + 1]
                    )
            nc.sync.dma_start(
                out=out_cat[ch * 2048 : (ch + 1) * 2048, :].rearrange(
                    "(t p) d -> p t d", p=128
                ),
                in_=y_all,
            )
    nc.compile()
    return nc


def _prep_host(inputs):
    f = lambda x: np.ascontiguousarray(np.asarray(x, dtype=np.float32))
    wm = {}
    wm["wq_ads"] = f(np.asarray(inputs["Wq_ads"]).T * SCALE)
    wm["bq_ads"] = f(np.asarray(inputs["bq_ads"]) * SCALE)
    wm["wk_ads"] = f(np.asarray(inputs["Wk_ads"]).T)
    wm["bk_ads"] = f(inputs["bk_ads"])
    wm["wv_ads"] = f(np.asarray(inputs["Wv_ads"]).T)
    wm["bv_ads"] = f(inputs["bv_ads"])
    wm["wq_cat"] = f(np.asarray(inputs["Wq_cat"]).T * SCALE)
    wm["bq_cat"] = f(np.asarray(inputs["bq_cat"]) * SCALE)
    wm["wk_cat"] = f(np.asarray(inputs["Wk_cat"]).T)
    wm["bk_cat"] = f(inputs["bk_cat"])
    wm["wv_cat"] = f(np.asarray(inputs["Wv_cat"]).T)
    wm["bv_cat"] = f(inputs["bv_cat"])
    wm["ident"] = np.eye(128, dtype=np.float32)
    return wm


DUMPS = None


def _install_profile_hook():
    """Provide antenv.axon_hooks + the ctypes NTFF hook this container lacks."""
    import sys, types, ctypes, contextlib

    if "antenv.axon_hooks" in sys.modules:
        return
    mod = types.ModuleType("antenv.axon_hooks")
    state = {"hook": None}
    mod.set_axon_ntff_profile_hook = lambda h: state.__setitem__("hook", h)
    mod.get_axon_ntff_profile_hook = lambda: state["hook"]
    sys.modules["antenv.axon_hooks"] = mod
    import antenv
    antenv.axon_hooks = mod

    lib = ctypes.CDLL("/opt/axon/libaxon_pjrt.so")
    if not hasattr(lib, "axon_start_nrt_profile"):
        return
    lib.axon_start_nrt_profile.argtypes = [ctypes.POINTER(ctypes.c_int64), ctypes.c_size_t]
    lib.axon_start_nrt_profile.restype = ctypes.c_int64
    lib.axon_stop_nrt_profile.argtypes = [ctypes.c_char_p]
    lib.axon_stop_nrt_profile.restype = ctypes.c_int64

    @contextlib.contextmanager
    def _hook(output_dir, device_ids):
        import jax
        jax.devices()
        if device_ids:
            ids = (ctypes.c_int64 * len(device_ids))(*device_ids)
            rc = lib.axon_start_nrt_profile(ids, len(device_ids))
        else:
            rc = lib.axon_start_nrt_profile(None, 0)
        if rc != 0:
            raise RuntimeError(f"axon_start_nrt_profile rc={rc}")
        try:
            yield
        finally:
            n = lib.axon_stop_nrt_profile(str(output_dir).encode())
            print(f"profile: {n} file(s) written to {output_dir}")

    mod.set_axon_ntff_profile_hook(_hook)
    # avoid bucket uploads from this container
    import concourse.bass_utils as _bu
    _bu.upload_artifacts = lambda d: str(d)


def kernel(trace=False, g_core=G_CORE, dbg=False, **inputs):
    global LAST_EXEC_NS, DUMPS
    key = (g_core, dbg)
    if key not in _CACHE:
        _CACHE[key] = build_nc(g_core, dbg)
    nc = _CACHE[key]
    if trace:
        _install_profile_hook()

    H_ads = np.ascontiguousarray(np.asarray(inputs["h_ads"], dtype=np.float32))
    H_cat = np.ascontiguousarray(np.asarray(inputs["h_cat"], dtype=np.float32))
    wm = _prep_host(inputs)

    na_core, nc_core = g_core * NA, g_core * NC
    in_maps = []
    for c in range(N_CORES):
        m = dict(wm)
        m["h_ads"] = H_ads[c * na_core : (c + 1) * na_core]
        m["h_cat"] = H_cat[c * nc_core : (c + 1) * nc_core]
        m["h_cat_bf"] = np.ascontiguousarray(m["h_cat"].astype(ml_dtypes.bfloat16))
        m["h_ads_bf"] = np.ascontiguousarray(m["h_ads"].astype(ml_dtypes.bfloat16))
        in_maps.append(m)

    res = run_bass_kernel_spmd(nc, in_maps, list(range(N_CORES)), trace=trace)
    LAST_EXEC_NS = getattr(res, "exec_time_ns", None)
    outs = res.results
    DUMPS = outs
    new_h_ads = np.concatenate([outs[c]["out"][:na_core] for c in range(N_CORES)], axis=0)
    new_h_cat = np.concatenate([outs[c]["out"][na_core:] for c in range(N_CORES)], axis=0)
    return new_h_ads, new_h_cat


def bench(iters=8, g_core=G_CORE, **inputs):
    """Run once for outputs + time repeated executions of the compiled NEFF."""
    global LAST_EXEC_NS
    import time
    import jax
    from jax.sharding import Mesh, PartitionSpec
    from jax.experimental.shard_map import shard_map
    from concourse import bass2jax, mybir as _mb

    key = (g_core, False)
    if key not in _CACHE:
        _CACHE[key] = build_nc(g_core, False)
    nc = _CACHE[key]
    bass2jax.install_neuronx_cc_hook()

    H_ads = np.ascontiguousarray(np.asarray(inputs["h_ads"], dtype=np.float32))
    H_cat = np.ascontiguousarray(np.asarray(inputs["h_cat"], dtype=np.float32))
    wm = _prep_host(inputs)
    na_core, nc_core = g_core * NA, g_core * NC
    in_maps = []
    for c in range(N_CORES):
        m = dict(wm)
        m["h_ads"] = H_ads[c * na_core : (c + 1) * na_core]
        m["h_cat"] = H_cat[c * nc_core : (c + 1) * nc_core]
        m["h_cat_bf"] = np.ascontiguousarray(m["h_cat"].astype(ml_dtypes.bfloat16))
        m["h_ads_bf"] = np.ascontiguousarray(m["h_ads"].astype(ml_dtypes.bfloat16))
        in_maps.append(m)

    partition_name = nc.partition_id_tensor.name if nc.partition_id_tensor else None
    in_names, out_names, out_avals, zero_outs = [], [], [], []
    for alloc in nc.m.functions[0].allocations:
        if not isinstance(alloc, _mb.MemoryLocationSet):
            continue
        name = alloc.memorylocations[0].name
        if alloc.kind == "ExternalInput":
            if name != partition_name:
                in_names.append(name)
        elif alloc.kind == "ExternalOutput":
            shape = tuple(alloc.tensor_shape)
            dtype = _mb.dt.np(alloc.dtype)
            out_avals.append(jax.core.ShapedArray(shape, dtype))
            out_names.append(name)
            zero_outs.append(np.zeros(shape, dtype))
    n_params, n_outs = len(in_names), len(out_avals)
    in_names_all = in_names + out_names + ([partition_name] if partition_name else [])

    def _body(*args):
        operands = list(args)
        if partition_name is not None:
            operands.append(bass2jax.partition_id_tensor())
        outs = bass2jax._bass_exec_p.bind(
            *operands,
            out_avals=tuple(out_avals),
            in_names=tuple(in_names_all),
            out_names=tuple(out_names),
            lowering_input_output_aliases=(),
            sim_require_finite=True,
            sim_require_nnan=True,
            nc=nc,
        )
        return tuple(outs)

    devices = jax.devices()[:N_CORES]
    mesh = Mesh(np.asarray(devices), ("core",))
    donate = tuple(range(n_params, n_params + n_outs))
    sharded = jax.jit(
        shard_map(_body, mesh=mesh,
                  in_specs=(PartitionSpec("core"),) * (n_params + n_outs),
                  out_specs=(PartitionSpec("core"),) * n_outs, check_rep=False),
        keep_unused=True,
    )
    concat_in = [np.concatenate([np.asarray(in_maps[c][k]) for c in range(N_CORES)], axis=0)
                 for k in in_names]
    concat_zero = [np.zeros((N_CORES * z.shape[0], *z.shape[1:]), z.dtype) for z in zero_outs]
    from jax.sharding import NamedSharding
    shard = NamedSharding(mesh, PartitionSpec("core"))
    dev_in = [jax.device_put(a, shard) for a in concat_in]

    # warmup/compile + correctness outputs
    out_arrs = sharded(*dev_in, *[jax.device_put(z, shard) for z in concat_zero])
    del donate
    jax.block_until_ready(out_arrs)
    out0 = np.asarray(out_arrs[0]).reshape(N_CORES, -1, D)

    zs = [jax.device_put(z, shard) for z in concat_zero]
    jax.block_until_ready(zs)
    times = []
    for _ in range(iters):
        t0 = time.perf_counter()
        o = sharded(*dev_in, *zs)
        jax.block_until_ready(o)
        times.append(time.perf_counter() - t0)
    LAST_EXEC_NS = int(min(times) * 1e9)

    new_h_ads = np.concatenate([out0[c][:na_core] for c in range(N_CORES)], axis=0)
    new_h_cat = np.concatenate([out0[c][na_core:] for c in range(N_CORES)], axis=0)
    return new_h_ads, new_h_cat, LAST_EXEC_NS, times
